# revision 5
# baseline (speedup 1.0000x reference)
"""MultiHeadDualAttention Trainium2 kernel.

Sharding: 8 heads -> 8 cores (tensor parallel over heads). Each core gets the
full k1/v1/k2/v2 (pre-transposed on host to [256, 4096] so the contraction dim
lands on SBUF partitions) plus its head's slices of the wk/wv/wo weights.

Math per head (verified exact vs reference in fp64):
  o2 = rowsoftmax(S_true) @ v2p_full ; o1 = colsoftmax(S_true)^T @ v1p_full
  - v-bias cancels through softmax row-sums == 1, re-added on host via
    (bv @ wo + bo) constants.
  - k-bias: rowsoftmax(S_true) == rowsoftmax(k1p_FULL @ k2p_NOB^T) and
    colsoftmax(S_true) == colsoftmax(k1p_NOB @ k2p_FULL^T), so each direction
    uses one biased and one unbiased projection and no rank-1 corrections.
  - exp without max-subtraction: |SCALE*S| < ~2.5, safe in fp32.
  - softmax denominators exported unnormalized (den1/den2); host divides.

Each direction is one pass producing E[part, free] = exp(SCALE * kP^T kF) in
32x[128, 512] SBUF tiles per 512-wide free-block, then PV accumulation with a
ones-augmented V to get [65, 512] (64 output dims + denominator row) in PSUM.
Output projection wo is applied on-device per head ([64,256] slice); host sums
the 8 partial [256, 4096] results (the "all-reduce" of the row-sharded wo).
"""

import sys

sys.path.insert(0, "/opt/trn_rl_repo")

import numpy as np

N = 4096
C = 256
AD = 512
H = 8
D = 64
SCALE = float(D) ** -0.5
NCORES = 8
NBLK = 512          # free-dim block width for E
NCHUNK = N // NBLK  # 8 blocks
MT = N // 128       # 32 partition-tiles of E per block

_cache: dict = {}


def _build_module():
    import concourse.bacc as bacc
    import concourse.mybir as mybir
    import concourse.tile as tile

    f32 = mybir.dt.float32
    f32r = mybir.dt.float32r
    bf16 = mybir.dt.bfloat16
    Exp = mybir.ActivationFunctionType.Exp

    nc = bacc.Bacc("TRN2", target_bir_lowering=False, debug=False)

    def din(name, shape, dt=bf16):
        return nc.dram_tensor(name, shape, dt, kind="ExternalInput").ap()

    def dout(name, shape):
        return nc.dram_tensor(name, shape, f32, kind="ExternalOutput").ap()

    k1T = din("k1T", [C, N])
    v1T = din("v1T", [C, N])
    k2T = din("k2T", [C, N])
    v2T = din("v2T", [C, N])
    wk1 = din("wk1", [C, D])
    wv1 = din("wv1", [C, D])
    wk2 = din("wk2", [C, D])
    wv2 = din("wv2", [C, D])
    bk1 = din("bk1", [D, 1], f32)
    bk2 = din("bk2", [D, 1], f32)
    wo1 = din("wo1", [D, C])
    wo2 = din("wo2", [D, C])

    o1pT = dout("o1pT", [C, N])
    o2pT = dout("o2pT", [C, N])
    den1 = dout("den1", [1, N])
    den2 = dout("den2", [1, N])

    with tile.TileContext(nc) as tc:
        with (
            tc.tile_pool(name="const", bufs=1) as constp,
            tc.tile_pool(name="raw", bufs=3) as rawp,
            tc.tile_pool(name="eblk", bufs=2) as ep,
            tc.tile_pool(name="outp", bufs=3) as outp,
            tc.tile_pool(name="spsum", bufs=3, space="PSUM") as spsum,
            tc.tile_pool(name="opsum", bufs=2, space="PSUM") as opsum,
            tc.tile_pool(name="ppsum", bufs=2, space="PSUM") as ppsum,
        ):
            # ---- load weights ----
            wk_sb = {}
            for name, drt in (("wk1", wk1), ("wv1", wv1), ("wk2", wk2), ("wv2", wv2)):
                t = constp.tile([128, 2, D], bf16, tag=name)
                for ct in range(2):
                    nc.sync.dma_start(out=t[:, ct, :], in_=drt[ct * 128:(ct + 1) * 128, :])
                wk_sb[name] = t
            bk1_sb = constp.tile([D, 1], f32, tag="bk1")
            nc.sync.dma_start(out=bk1_sb[:], in_=bk1[:])
            bk2_sb = constp.tile([D, 1], f32, tag="bk2")
            nc.sync.dma_start(out=bk2_sb[:], in_=bk2[:])
            wo1_sb = constp.tile([D, C], bf16, tag="wo1")
            nc.sync.dma_start(out=wo1_sb[:], in_=wo1[:])
            wo2_sb = constp.tile([D, C], bf16, tag="wo2")
            nc.sync.dma_start(out=wo2_sb[:], in_=wo2[:])

            # ---- k projections: [64, 4096] nob + full variants ----
            def k_proj(rawT, w_sb, b_sb, tagbase):

                nob = constp.tile([D, N], bf16, tag=tagbase + "_nob")
                full = constp.tile([D, N], bf16, tag=tagbase + "_full")
                for j in range(NCHUNK):
                    raw = rawp.tile([128, 2, NBLK], bf16, tag="raw")
                    for ct in range(2):
                        nc.sync.dma_start(
                            out=raw[:, ct, :],
                            in_=rawT[ct * 128:(ct + 1) * 128, j * NBLK:(j + 1) * NBLK],
                        )
                    ps = ppsum.tile([D, NBLK], f32, tag="pp")
                    for ct in range(2):
                        nc.tensor.matmul(
                            ps[:], w_sb[:, ct, :], raw[:, ct, :],
                            start=(ct == 0), stop=(ct == 1),
                        )
                    nc.vector.tensor_copy(nob[:, j * NBLK:(j + 1) * NBLK], ps[:])
                nc.vector.tensor_scalar_add(full[:], nob[:], b_sb[:])
                return nob, full

            k1_nob, k1_full = k_proj(k1T, wk_sb["wk1"], bk1_sb, "k1p")
            k2_nob, k2_full = k_proj(k2T, wk_sb["wk2"], bk2_sb, "k2p")

            # ---- v projections: [128, 32, 65] bf16, ones in col 64 ----
            def v_proj(rawT, w_sb, tagbase):
                vaug = constp.tile([128, MT, D + 1], bf16, tag=tagbase)
                nc.vector.memset(vaug[:, :, D:D + 1], 1.0)
                for j in range(NCHUNK):
                    raw = rawp.tile([128, 2, NBLK], bf16, tag="raw")
                    for ct in range(2):
                        nc.sync.dma_start(
                            out=raw[:, ct, :],
                            in_=rawT[ct * 128:(ct + 1) * 128, j * NBLK:(j + 1) * NBLK],
                        )
                    for k in range(NBLK // 128):
                        nt = j * (NBLK // 128) + k
                        ps = ppsum.tile([128, D], f32, tag="pp")
                        for ct in range(2):
                            nc.tensor.matmul(
                                ps[:], raw[:, ct, k * 128:(k + 1) * 128],
                                w_sb[:, ct, :],
                                start=(ct == 0), stop=(ct == 1),
                            )
                        nc.vector.tensor_copy(vaug[:, nt, :D], ps[:])
                return vaug

            v1_aug = v_proj(v1T, wk_sb["wv1"], "v1aug")
            v2_aug = v_proj(v2T, wk_sb["wv2"], "v2aug")

            # ---- one softmax direction ----
            def attention_pass(kP, kF, vaug, oT_tag):
                """E[p, f] = exp(SCALE * kP[:,p]^T kF[:,f]); oT = [vaug|1]^T E."""
                oT = constp.tile([D, N], bf16, tag=oT_tag)
                dsb = constp.tile([1, N], f32, tag=oT_tag + "_den")
                for j in range(NCHUNK):
                    eblk = ep.tile([128, MT, NBLK], bf16, tag="eblk")
                    for mt in range(MT):
                        ps = spsum.tile([128, NBLK], f32, tag="sp")
                        nc.tensor.matmul(
                            ps[:], kP[:, mt * 128:(mt + 1) * 128],
                            kF[:, j * NBLK:(j + 1) * NBLK],
                            start=True, stop=True,
                        )
                        nc.scalar.activation(eblk[:, mt, :], ps[:], Exp, scale=SCALE)
                    po = opsum.tile([D + 1, NBLK], f32, tag="op")
                    for mt in range(MT):
                        nc.tensor.matmul(
                            po[:], vaug[:, mt, :], eblk[:, mt, :],
                            start=(mt == 0), stop=(mt == MT - 1),
                        )
                    nc.vector.tensor_copy(oT[:, j * NBLK:(j + 1) * NBLK], po[0:D, :])
                    nc.vector.tensor_copy(dsb[:, j * NBLK:(j + 1) * NBLK], po[D:D + 1, :])
                return oT, dsb

            # o2: E[m, n] from k2p_nob (partition) x k1p_full (free)
            o2T, d2sb = attention_pass(k2_nob, k1_full, v2_aug, "o2T")
            # o1: E[n, m] from k1p_nob (partition) x k2p_full (free)
            o1T, d1sb = attention_pass(k1_nob, k2_full, v1_aug, "o1T")

            # ---- output projections (per-head slice of wo) ----
            def out_proj(oT, dsb, wo_sb, outdr, dendr):
                for ct in range(2):
                    for j in range(NCHUNK):
                        pp = ppsum.tile([128, NBLK], f32, tag="pp")
                        nc.tensor.matmul(
                            pp[:], wo_sb[:, ct * 128:(ct + 1) * 128],
                            oT[:, j * NBLK:(j + 1) * NBLK],
                            start=True, stop=True,
                        )
                        ot = outp.tile([128, NBLK], f32, tag="out")
                        nc.vector.tensor_copy(ot[:], pp[:])
                        nc.sync.dma_start(
                            out=outdr[ct * 128:(ct + 1) * 128, j * NBLK:(j + 1) * NBLK],
                            in_=ot[:],
                        )
                nc.sync.dma_start(out=dendr[:], in_=dsb[:])

            out_proj(o2T, d2sb, wo2_sb, o2pT, den2)
            out_proj(o1T, d1sb, wo1_sb, o1pT, den1)

    nc.compile()
    return nc


def _get_nc():
    if "nc" not in _cache:
        _cache["nc"] = _build_module()
    return _cache["nc"]


def kernel(k1, v1, k2, v2,
           wk1_w, wk1_b, wv1_w, wv1_b,
           wk2_w, wk2_b, wv2_w, wv2_b,
           wo1_w, wo1_b, wo2_w, wo2_b):
    import ml_dtypes
    from concourse.bass_utils import run_bass_kernel_spmd

    nc = _get_nc()

    f = np.float32
    bf = ml_dtypes.bfloat16
    k1T = np.ascontiguousarray(np.asarray(k1, f).T).astype(bf)
    v1T = np.ascontiguousarray(np.asarray(v1, f).T).astype(bf)
    k2T = np.ascontiguousarray(np.asarray(k2, f).T).astype(bf)
    v2T = np.ascontiguousarray(np.asarray(v2, f).T).astype(bf)

    in_maps = []
    for h in range(NCORES):
        sl = slice(h * D, (h + 1) * D)
        in_maps.append({
            "k1T": k1T, "v1T": v1T, "k2T": k2T, "v2T": v2T,
            "wk1": np.ascontiguousarray(np.asarray(wk1_w, f)[:, sl]).astype(bf),
            "wv1": np.ascontiguousarray(np.asarray(wv1_w, f)[:, sl]).astype(bf),
            "wk2": np.ascontiguousarray(np.asarray(wk2_w, f)[:, sl]).astype(bf),
            "wv2": np.ascontiguousarray(np.asarray(wv2_w, f)[:, sl]).astype(bf),
            "bk1": np.ascontiguousarray(np.asarray(wk1_b, f)[sl].reshape(D, 1)),
            "bk2": np.ascontiguousarray(np.asarray(wk2_b, f)[sl].reshape(D, 1)),
            "wo1": np.ascontiguousarray(np.asarray(wo1_w, f)[sl, :]).astype(bf),
            "wo2": np.ascontiguousarray(np.asarray(wo2_w, f)[sl, :]).astype(bf),
        })

    res = run_bass_kernel_spmd(nc, in_maps, list(range(NCORES)))
    _cache["last_result"] = res

    out1 = np.zeros((N, C), np.float32)
    out2 = np.zeros((N, C), np.float32)
    for h in range(NCORES):
        rh = res.results[h]
        out1 += (rh["o1pT"] / rh["den1"]).T
        out2 += (rh["o2pT"] / rh["den2"]).T
    # v-bias and output bias constants (v-bias commutes through softmax).
    out1 += np.asarray(wv1_b, f) @ np.asarray(wo1_w, f) + np.asarray(wo1_b, f)
    out2 += np.asarray(wv2_b, f) @ np.asarray(wo2_w, f) + np.asarray(wo2_b, f)
    return out1, out2


# revision 7
# speedup vs baseline: 1.0670x; 1.0670x over previous
"""MultiHeadDualAttention Trainium2 kernel.

Sharding: 8 heads -> 8 cores (tensor parallel over heads). Each core gets the
full k1/v1/k2/v2 (pre-transposed on host to [256, 4096] so the contraction dim
lands on SBUF partitions) plus its head's slices of the wk/wv/wo weights.

Math per head (verified exact vs reference in fp64):
  o2 = rowsoftmax(S_true) @ v2p_full ; o1 = colsoftmax(S_true)^T @ v1p_full
  - v-bias cancels through softmax row-sums == 1, re-added on host via
    (bv @ wo + bo) constants.
  - k-bias: rowsoftmax(S_true) == rowsoftmax(k1p_FULL @ k2p_NOB^T) and
    colsoftmax(S_true) == colsoftmax(k1p_NOB @ k2p_FULL^T), so each direction
    uses one biased and one unbiased projection and no rank-1 corrections.
  - exp without max-subtraction: |SCALE*S| < ~2.5, safe in fp32.
  - softmax denominators exported unnormalized (den1/den2); host divides.

Perf structure:
  - wk weights are shipped column-duplicated [256, 128] so the k projections
    land duplicated on both partition halves; the K=64 score matmuls then run
    2x row-packed (64x128 PE tiles T0/T8 via base_partition 0/64).
  - E is built in [128, 16, 1024] bf16 sub-blocks (exp at FD=1024 amortizes
    the ACT instruction overhead), double-buffered against the PV matmuls.
  - PV uses a ones-augmented V (M=65) accumulating [65, 512] in PSUM over all
    32 partition-tiles; row 64 is the softmax denominator.
Output projection wo is applied on-device per head ([64,256] slice); host sums
the 8 partial [256, 4096] results (the "all-reduce" of the row-sharded wo).
"""

import sys

sys.path.insert(0, "/opt/trn_rl_repo")

import numpy as np

N = 4096
C = 256
AD = 512
H = 8
D = 64
SCALE = float(D) ** -0.5
NCORES = 8
NBLK = 1024         # free-dim block width for E
NCHUNK = N // NBLK  # 4 blocks
MT = N // 128       # 32 partition-tiles of E per block
SUB = 16            # m-tiles per E sub-block

_cache: dict = {}


def _build_module():
    import concourse.bacc as bacc
    import concourse.mybir as mybir
    import concourse.tile as tile

    f32 = mybir.dt.float32
    bf16 = mybir.dt.bfloat16
    Exp = mybir.ActivationFunctionType.Exp

    nc = bacc.Bacc("TRN2", target_bir_lowering=False, debug=False)

    def din(name, shape, dt=bf16):
        return nc.dram_tensor(name, shape, dt, kind="ExternalInput").ap()

    def dout(name, shape):
        return nc.dram_tensor(name, shape, f32, kind="ExternalOutput").ap()

    k1T = din("k1T", [C, N])
    v1T = din("v1T", [C, N])
    k2T = din("k2T", [C, N])
    v2T = din("v2T", [C, N])
    wk1 = din("wk1", [C, 128])   # column-duplicated [wk|wk]
    wk2 = din("wk2", [C, 128])
    wv1 = din("wv1", [C, D])
    wv2 = din("wv2", [C, D])
    bk1 = din("bk1", [128, 1], f32)  # row-duplicated
    bk2 = din("bk2", [128, 1], f32)
    wo1 = din("wo1", [D, C])
    wo2 = din("wo2", [D, C])

    o1pT = dout("o1pT", [C, N])
    o2pT = dout("o2pT", [C, N])
    den1 = dout("den1", [1, N])
    den2 = dout("den2", [1, N])

    with tile.TileContext(nc) as tc:
        with (
            tc.tile_pool(name="const", bufs=1) as constp,
            tc.tile_pool(name="raw", bufs=3) as rawp,
            tc.tile_pool(name="eblk", bufs=2) as ep,
            tc.tile_pool(name="outp", bufs=3) as outp,
            tc.tile_pool(name="spsum", bufs=3, space="PSUM") as spsum,
            tc.tile_pool(name="opsum", bufs=2, space="PSUM") as opsum,
        ):
            # ---- load weights ----
            w_sb = {}
            for name, drt, w in (("wk1", wk1, 128), ("wk2", wk2, 128),
                                 ("wv1", wv1, D), ("wv2", wv2, D)):
                t = constp.tile([128, 2, w], bf16, tag=name)
                for ct in range(2):
                    nc.sync.dma_start(out=t[:, ct, :], in_=drt[ct * 128:(ct + 1) * 128, :])
                w_sb[name] = t
            bk1_sb = constp.tile([128, 1], f32, tag="bk1")
            nc.sync.dma_start(out=bk1_sb[:], in_=bk1[:])
            bk2_sb = constp.tile([128, 1], f32, tag="bk2")
            nc.sync.dma_start(out=bk2_sb[:], in_=bk2[:])
            wo1_sb = constp.tile([D, C], bf16, tag="wo1")
            nc.sync.dma_start(out=wo1_sb[:], in_=wo1[:])
            wo2_sb = constp.tile([D, C], bf16, tag="wo2")
            nc.sync.dma_start(out=wo2_sb[:], in_=wo2[:])

            # ---- k projections: [128, 4096] bf16, data duplicated on both
            # partition halves (weights are column-duplicated) ----
            def k_proj(rawT, w, b_sb, tagbase):
                nob = constp.tile([128, N], bf16, tag=tagbase + "_nob")
                full = constp.tile([128, N], bf16, tag=tagbase + "_full")
                for j in range(8):
                    raw = rawp.tile([128, 2, 512], bf16, tag="raw")
                    for ct in range(2):
                        nc.sync.dma_start(
                            out=raw[:, ct, :],
                            in_=rawT[ct * 128:(ct + 1) * 128, j * 512:(j + 1) * 512],
                        )
                    ps = opsum.tile([128, 512], f32, tag="op")
                    for ct in range(2):
                        nc.tensor.matmul(
                            ps[:], w[:, ct, :], raw[:, ct, :],
                            start=(ct == 0), stop=(ct == 1),
                        )
                    nc.vector.tensor_copy(nob[:, j * 512:(j + 1) * 512], ps[:])
                nc.vector.tensor_scalar_add(full[:], nob[:], b_sb[:])
                return nob, full

            k1_nob, k1_full = k_proj(k1T, w_sb["wk1"], bk1_sb, "k1p")
            k2_nob, k2_full = k_proj(k2T, w_sb["wk2"], bk2_sb, "k2p")

            # ---- v projections: [128, 32, 65] bf16, ones in col 64 ----
            def v_proj(rawT, w, tagbase):
                vaug = constp.tile([128, MT, D + 1], bf16, tag=tagbase)
                nc.vector.memset(vaug[:, :, D:D + 1], 1.0)
                for j in range(8):
                    raw = rawp.tile([128, 2, 512], bf16, tag="raw")
                    for ct in range(2):
                        nc.sync.dma_start(
                            out=raw[:, ct, :],
                            in_=rawT[ct * 128:(ct + 1) * 128, j * 512:(j + 1) * 512],
                        )
                    for k in range(4):
                        nt = j * 4 + k
                        ps = opsum.tile([128, D], f32, tag="op")
                        for ct in range(2):
                            nc.tensor.matmul(
                                ps[:], raw[:, ct, k * 128:(k + 1) * 128],
                                w[:, ct, :],
                                start=(ct == 0), stop=(ct == 1),
                            )
                        nc.vector.tensor_copy(vaug[:, nt, :D], ps[:])
                return vaug

            v1_aug = v_proj(v1T, w_sb["wv1"], "v1aug")
            v2_aug = v_proj(v2T, w_sb["wv2"], "v2aug")

            # ---- one softmax direction ----
            def attention_pass(kP, kF, vaug, oT_tag):
                """E[p, f] = exp(SCALE * kP[:,p]^T kF[:,f]); oT = [vaug|1]^T E.

                kP/kF are partition-duplicated [128, N]; score matmuls run as
                2x row-packed 64x128 PE tiles (T0 rows 0-63, T8 rows 64-127).
                """
                oT = constp.tile([D, N], bf16, tag=oT_tag)
                dsb = constp.tile([1, N], f32, tag=oT_tag + "_den")
                for j in range(NCHUNK):
                    po = [opsum.tile([D + 1, 512], f32, tag="op", name=f"po_{oT_tag}_{j}_{c}")
                          for c in range(2)]
                    for sub in range(2):
                        eblk = ep.tile([128, SUB, NBLK], bf16, tag="eblk")
                        for pair in range(SUB // 2):
                            mtA = sub * SUB + 2 * pair
                            mtB = mtA + 1
                            psA = spsum.tile([128, NBLK], f32, tag="sp")
                            psB = spsum.tile([128, NBLK], f32, tag="sp")
                            for ps, mt, lo, hi in ((psA, mtA, 0, 64), (psB, mtB, 64, 128)):
                                for c in range(2):
                                    nc.tensor.matmul(
                                        ps[:, c * 512:(c + 1) * 512],
                                        kP[lo:hi, mt * 128:(mt + 1) * 128],
                                        kF[lo:hi, j * NBLK + c * 512: j * NBLK + (c + 1) * 512],
                                        start=True, stop=True,
                                    )
                            nc.scalar.activation(eblk[:, 2 * pair, :], psA[:], Exp, scale=SCALE)
                            nc.scalar.activation(eblk[:, 2 * pair + 1, :], psB[:], Exp, scale=SCALE)
                        for c in range(2):
                            for mtl in range(SUB):
                                mt = sub * SUB + mtl
                                nc.tensor.matmul(
                                    po[c][:], vaug[:, mt, :],
                                    eblk[:, mtl, c * 512:(c + 1) * 512],
                                    start=(mt == 0), stop=(mt == MT - 1),
                                )
                    for c in range(2):
                        base = j * NBLK + c * 512
                        nc.vector.tensor_copy(oT[:, base:base + 512], po[c][0:D, :])
                        nc.vector.tensor_copy(dsb[:, base:base + 512], po[c][D:D + 1, :])
                return oT, dsb

            # o2: E[m, n] from k2p_nob (partition) x k1p_full (free)
            o2T, d2sb = attention_pass(k2_nob, k1_full, v2_aug, "o2T")
            # o1: E[n, m] from k1p_nob (partition) x k2p_full (free)
            o1T, d1sb = attention_pass(k1_nob, k2_full, v1_aug, "o1T")

            # ---- output projections (per-head slice of wo) ----
            def out_proj(oT, dsb, wo_sb, outdr, dendr):
                for ct in range(2):
                    for j in range(8):
                        pp = opsum.tile([128, 512], f32, tag="op")
                        nc.tensor.matmul(
                            pp[:], wo_sb[:, ct * 128:(ct + 1) * 128],
                            oT[:, j * 512:(j + 1) * 512],
                            start=True, stop=True,
                        )
                        ot = outp.tile([128, 512], f32, tag="out")
                        nc.vector.tensor_copy(ot[:], pp[:])
                        nc.sync.dma_start(
                            out=outdr[ct * 128:(ct + 1) * 128, j * 512:(j + 1) * 512],
                            in_=ot[:],
                        )
                nc.sync.dma_start(out=dendr[:], in_=dsb[:])

            out_proj(o2T, d2sb, wo2_sb, o2pT, den2)
            out_proj(o1T, d1sb, wo1_sb, o1pT, den1)

    nc.compile()
    return nc


def _get_nc():
    if "nc" not in _cache:
        _cache["nc"] = _build_module()
    return _cache["nc"]


def kernel(k1, v1, k2, v2,
           wk1_w, wk1_b, wv1_w, wv1_b,
           wk2_w, wk2_b, wv2_w, wv2_b,
           wo1_w, wo1_b, wo2_w, wo2_b):
    import ml_dtypes
    from concourse.bass_utils import run_bass_kernel_spmd

    nc = _get_nc()

    f = np.float32
    bf = ml_dtypes.bfloat16
    k1T = np.ascontiguousarray(np.asarray(k1, f).T).astype(bf)
    v1T = np.ascontiguousarray(np.asarray(v1, f).T).astype(bf)
    k2T = np.ascontiguousarray(np.asarray(k2, f).T).astype(bf)
    v2T = np.ascontiguousarray(np.asarray(v2, f).T).astype(bf)

    def dup2(a):  # [C, D] -> [C, 128] column-duplicated
        return np.ascontiguousarray(np.concatenate([a, a], axis=1))

    in_maps = []
    for h in range(NCORES):
        sl = slice(h * D, (h + 1) * D)
        in_maps.append({
            "k1T": k1T, "v1T": v1T, "k2T": k2T, "v2T": v2T,
            "wk1": dup2(np.asarray(wk1_w, f)[:, sl]).astype(bf),
            "wv1": np.ascontiguousarray(np.asarray(wv1_w, f)[:, sl]).astype(bf),
            "wk2": dup2(np.asarray(wk2_w, f)[:, sl]).astype(bf),
            "wv2": np.ascontiguousarray(np.asarray(wv2_w, f)[:, sl]).astype(bf),
            "bk1": np.ascontiguousarray(np.tile(np.asarray(wk1_b, f)[sl].reshape(D, 1), (2, 1))),
            "bk2": np.ascontiguousarray(np.tile(np.asarray(wk2_b, f)[sl].reshape(D, 1), (2, 1))),
            "wo1": np.ascontiguousarray(np.asarray(wo1_w, f)[sl, :]).astype(bf),
            "wo2": np.ascontiguousarray(np.asarray(wo2_w, f)[sl, :]).astype(bf),
        })

    res = run_bass_kernel_spmd(nc, in_maps, list(range(NCORES)))
    _cache["last_result"] = res

    out1 = np.zeros((N, C), np.float32)
    out2 = np.zeros((N, C), np.float32)
    for h in range(NCORES):
        rh = res.results[h]
        out1 += (rh["o1pT"] / rh["den1"]).T
        out2 += (rh["o2pT"] / rh["den2"]).T
    # v-bias and output bias constants (v-bias commutes through softmax).
    out1 += np.asarray(wv1_b, f) @ np.asarray(wo1_w, f) + np.asarray(wo1_b, f)
    out2 += np.asarray(wv2_b, f) @ np.asarray(wo2_w, f) + np.asarray(wo2_b, f)
    return out1, out2


# revision 10
# speedup vs baseline: 1.0907x; 1.0222x over previous
"""MultiHeadDualAttention Trainium2 kernel.

Sharding: 8 heads -> 8 cores (tensor parallel over heads). Each core gets the
full k1/v1/k2/v2 (pre-transposed on host to [256, 4096] so the contraction dim
lands on SBUF partitions) plus its head's slices of the wk/wv/wo weights.

Math per head (verified exact vs reference in fp64):
  o2 = rowsoftmax(S_true) @ v2p_full ; o1 = colsoftmax(S_true)^T @ v1p_full
  - v-bias cancels through softmax row-sums == 1, re-added on host via
    (bv @ wo + bo) constants.
  - k-bias: rowsoftmax(S_true) == rowsoftmax(k1p_FULL @ k2p_NOB^T) and
    colsoftmax(S_true) == colsoftmax(k1p_NOB @ k2p_FULL^T), so each direction
    uses one biased and one unbiased projection and no rank-1 corrections.
  - exp without max-subtraction: |SCALE*S| < ~2.5, safe in fp32.
  - softmax denominators exported unnormalized (den1/den2); host divides.

Perf structure:
  - wk weights are shipped column-duplicated [256, 128] so the k projections
    land duplicated on both partition halves; the K=64 score matmuls then run
    2x row-packed (64x128 PE tiles T0/T8 via base_partition 0/64).
  - E is built in [128, 16, 1024] bf16 sub-blocks (exp at FD=1024 amortizes
    the ACT instruction overhead), double-buffered against the PV matmuls.
  - PV uses a ones-augmented V (M=65) accumulating [65, 512] in PSUM over all
    32 partition-tiles; row 64 is the softmax denominator.
Output projection wo is applied on-device per head ([64,256] slice); host sums
the 8 partial [256, 4096] results (the "all-reduce" of the row-sharded wo).
"""

import sys

sys.path.insert(0, "/opt/trn_rl_repo")

import numpy as np

N = 4096
C = 256
AD = 512
H = 8
D = 64
SCALE = float(D) ** -0.5
NCORES = 8
NBLK = 1024         # free-dim block width for E
NCHUNK = N // NBLK  # 4 blocks
MT = N // 128       # 32 partition-tiles of E per block
SUB = 16            # m-tiles per E sub-block

_cache: dict = {}


def _build_module():
    import concourse.bacc as bacc
    import concourse.mybir as mybir
    import concourse.tile as tile

    f32 = mybir.dt.float32
    bf16 = mybir.dt.bfloat16
    Exp = mybir.ActivationFunctionType.Exp

    nc = bacc.Bacc("TRN2", target_bir_lowering=False, debug=False)

    def din(name, shape, dt=bf16):
        return nc.dram_tensor(name, shape, dt, kind="ExternalInput").ap()

    def dout(name, shape):
        return nc.dram_tensor(name, shape, f32, kind="ExternalOutput").ap()

    k1T = din("k1T", [C, N])
    v1T = din("v1T", [C, N])
    k2T = din("k2T", [C, N])
    v2T = din("v2T", [C, N])
    wk1 = din("wk1", [C, 128])   # column-duplicated [wk|wk]
    wk2 = din("wk2", [C, 128])
    wv1 = din("wv1", [C, D])
    wv2 = din("wv2", [C, D])
    bk1 = din("bk1", [128, 1], f32)  # row-duplicated
    bk2 = din("bk2", [128, 1], f32)
    wo1 = din("wo1", [D, C])
    wo2 = din("wo2", [D, C])

    o1pT = dout("o1pT", [C, N])
    o2pT = dout("o2pT", [C, N])
    den1 = dout("den1", [1, N])
    den2 = dout("den2", [1, N])

    with tile.TileContext(nc) as tc:
        with (
            tc.tile_pool(name="const", bufs=1) as constp,
            tc.tile_pool(name="raw", bufs=8) as rawp,
            tc.tile_pool(name="eblk", bufs=2) as ep,
            tc.tile_pool(name="outp", bufs=3) as outp,
            tc.tile_pool(name="spsum", bufs=3, space="PSUM") as spsum,
            tc.tile_pool(name="opsum", bufs=2, space="PSUM") as opsum,
        ):
            # ---- PE warm-up: ~7us of dummy matmuls so the HAM clock-gate
            # reaches K=8/8 (2.4 GHz) before the real work arrives ----
            warm = constp.tile([128, 512], bf16, tag="warm")
            nc.gpsimd.memset(warm[:], 0.0)
            wps = opsum.tile([128, 512], f32, tag="op", name="warm_ps")
            for _ in range(16):
                nc.tensor.matmul(wps[:], warm[:, 0:128], warm[:], start=True, stop=True)

            # ---- load weights ----
            w_sb = {}
            for name, drt, w in (("wk1", wk1, 128), ("wk2", wk2, 128),
                                 ("wv1", wv1, D), ("wv2", wv2, D)):
                t = constp.tile([128, 2, w], bf16, tag=name)
                for ct in range(2):
                    nc.sync.dma_start(out=t[:, ct, :], in_=drt[ct * 128:(ct + 1) * 128, :])
                w_sb[name] = t
            bk1_sb = constp.tile([128, 1], f32, tag="bk1")
            nc.sync.dma_start(out=bk1_sb[:], in_=bk1[:])
            bk2_sb = constp.tile([128, 1], f32, tag="bk2")
            nc.sync.dma_start(out=bk2_sb[:], in_=bk2[:])
            wo1_sb = constp.tile([D, C], bf16, tag="wo1")
            nc.sync.dma_start(out=wo1_sb[:], in_=wo1[:])
            wo2_sb = constp.tile([D, C], bf16, tag="wo2")
            nc.sync.dma_start(out=wo2_sb[:], in_=wo2[:])

            # ---- k projections: [128, 4096] bf16, data duplicated on both
            # partition halves (weights are column-duplicated) ----
            def k_proj(rawT, w, b_sb, tagbase):
                nob = constp.tile([128, N], bf16, tag=tagbase + "_nob")
                full = constp.tile([128, N], bf16, tag=tagbase + "_full")
                for j in range(8):
                    raw = rawp.tile([128, 2, 512], bf16, tag="raw")
                    for ct in range(2):
                        nc.sync.dma_start(
                            out=raw[:, ct, :],
                            in_=rawT[ct * 128:(ct + 1) * 128, j * 512:(j + 1) * 512],
                        )
                    ps = opsum.tile([128, 512], f32, tag="op")
                    for ct in range(2):
                        nc.tensor.matmul(
                            ps[:], w[:, ct, :], raw[:, ct, :],
                            start=(ct == 0), stop=(ct == 1),
                        )
                    nc.vector.tensor_copy(nob[:, j * 512:(j + 1) * 512], ps[:])
                nc.vector.tensor_scalar_add(full[:], nob[:], b_sb[:])
                return nob, full

            k1_nob, k1_full = k_proj(k1T, w_sb["wk1"], bk1_sb, "k1p")
            k2_nob, k2_full = k_proj(k2T, w_sb["wk2"], bk2_sb, "k2p")

            # ---- v projections: [128, 32, 65] bf16, ones in col 64 ----
            def v_proj(rawT, w, tagbase):
                vaug = constp.tile([128, MT, D + 1], bf16, tag=tagbase)
                nc.vector.memset(vaug[:, :, D:D + 1], 1.0)
                for j in range(8):
                    raw = rawp.tile([128, 2, 512], bf16, tag="raw")
                    for ct in range(2):
                        nc.sync.dma_start(
                            out=raw[:, ct, :],
                            in_=rawT[ct * 128:(ct + 1) * 128, j * 512:(j + 1) * 512],
                        )
                    for k in range(4):
                        nt = j * 4 + k
                        ps = opsum.tile([128, D], f32, tag="op")
                        for ct in range(2):
                            nc.tensor.matmul(
                                ps[:], raw[:, ct, k * 128:(k + 1) * 128],
                                w[:, ct, :],
                                start=(ct == 0), stop=(ct == 1),
                            )
                        nc.vector.tensor_copy(vaug[:, nt, :D], ps[:])
                return vaug

            v1_aug = v_proj(v1T, w_sb["wv1"], "v1aug")
            v2_aug = v_proj(v2T, w_sb["wv2"], "v2aug")

            # ---- one softmax direction ----
            def attention_pass(kP, kF, vaug, oT_tag):
                """E[p, f] = exp(SCALE * kP[:,p]^T kF[:,f]); oT = [vaug|1]^T E.

                kP/kF are partition-duplicated [128, N]; score matmuls run as
                2x row-packed 64x128 PE tiles (T0 rows 0-63, T8 rows 64-127).
                """
                oT = constp.tile([D, N], bf16, tag=oT_tag)
                dsb = constp.tile([1, N], f32, tag=oT_tag + "_den")
                for j in range(NCHUNK):
                    po = [opsum.tile([D + 1, 512], f32, tag="op", name=f"po_{oT_tag}_{j}_{c}")
                          for c in range(2)]
                    for sub in range(2):
                        eblk = ep.tile([128, SUB, NBLK], bf16, tag="eblk")
                        for pair in range(SUB // 2):
                            mtA = sub * SUB + 2 * pair
                            mtB = mtA + 1
                            psA = spsum.tile([128, NBLK], f32, tag="sp")
                            psB = spsum.tile([128, NBLK], f32, tag="sp")
                            for ps, mt, lo, hi in ((psA, mtA, 0, 64), (psB, mtB, 64, 128)):
                                for c in range(2):
                                    nc.tensor.matmul(
                                        ps[:, c * 512:(c + 1) * 512],
                                        kP[lo:hi, mt * 128:(mt + 1) * 128],
                                        kF[lo:hi, j * NBLK + c * 512: j * NBLK + (c + 1) * 512],
                                        start=True, stop=True,
                                    )
                            nc.scalar.activation(eblk[:, 2 * pair, :], psA[:], Exp, scale=SCALE)
                            nc.scalar.activation(eblk[:, 2 * pair + 1, :], psB[:], Exp, scale=SCALE)
                        for c in range(2):
                            for mtl in range(SUB):
                                mt = sub * SUB + mtl
                                nc.tensor.matmul(
                                    po[c][:], vaug[:, mt, :],
                                    eblk[:, mtl, c * 512:(c + 1) * 512],
                                    start=(mt == 0), stop=(mt == MT - 1),
                                )
                    for c in range(2):
                        base = j * NBLK + c * 512
                        nc.vector.tensor_copy(oT[:, base:base + 512], po[c][0:D, :])
                        nc.vector.tensor_copy(dsb[:, base:base + 512], po[c][D:D + 1, :])
                return oT, dsb

            # ---- output projections (per-head slice of wo) ----
            def out_proj(oT, dsb, wo_sb, outdr, dendr):
                for ct in range(2):
                    for j in range(8):
                        pp = opsum.tile([128, 512], f32, tag="op")
                        nc.tensor.matmul(
                            pp[:], wo_sb[:, ct * 128:(ct + 1) * 128],
                            oT[:, j * 512:(j + 1) * 512],
                            start=True, stop=True,
                        )
                        ot = outp.tile([128, 512], f32, tag="out")
                        nc.vector.tensor_copy(ot[:], pp[:])
                        nc.sync.dma_start(
                            out=outdr[ct * 128:(ct + 1) * 128, j * 512:(j + 1) * 512],
                            in_=ot[:],
                        )
                nc.sync.dma_start(out=dendr[:], in_=dsb[:])

            # o2: E[m, n] from k2p_nob (partition) x k1p_full (free)
            o2T, d2sb = attention_pass(k2_nob, k1_full, v2_aug, "o2T")
            # o2's projection interleaves into the o1 pass (PE gap-filler)
            out_proj(o2T, d2sb, wo2_sb, o2pT, den2)
            # o1: E[n, m] from k1p_nob (partition) x k2p_full (free)
            o1T, d1sb = attention_pass(k1_nob, k2_full, v1_aug, "o1T")
            out_proj(o1T, d1sb, wo1_sb, o1pT, den1)

    nc.compile()
    return nc


def _get_nc():
    if "nc" not in _cache:
        _cache["nc"] = _build_module()
    return _cache["nc"]


def kernel(k1, v1, k2, v2,
           wk1_w, wk1_b, wv1_w, wv1_b,
           wk2_w, wk2_b, wv2_w, wv2_b,
           wo1_w, wo1_b, wo2_w, wo2_b):
    import ml_dtypes
    from concourse.bass_utils import run_bass_kernel_spmd

    nc = _get_nc()

    f = np.float32
    bf = ml_dtypes.bfloat16
    k1T = np.ascontiguousarray(np.asarray(k1, f).T).astype(bf)
    v1T = np.ascontiguousarray(np.asarray(v1, f).T).astype(bf)
    k2T = np.ascontiguousarray(np.asarray(k2, f).T).astype(bf)
    v2T = np.ascontiguousarray(np.asarray(v2, f).T).astype(bf)

    def dup2(a):  # [C, D] -> [C, 128] column-duplicated
        return np.ascontiguousarray(np.concatenate([a, a], axis=1))

    in_maps = []
    for h in range(NCORES):
        sl = slice(h * D, (h + 1) * D)
        in_maps.append({
            "k1T": k1T, "v1T": v1T, "k2T": k2T, "v2T": v2T,
            "wk1": dup2(np.asarray(wk1_w, f)[:, sl]).astype(bf),
            "wv1": np.ascontiguousarray(np.asarray(wv1_w, f)[:, sl]).astype(bf),
            "wk2": dup2(np.asarray(wk2_w, f)[:, sl]).astype(bf),
            "wv2": np.ascontiguousarray(np.asarray(wv2_w, f)[:, sl]).astype(bf),
            "bk1": np.ascontiguousarray(np.tile(np.asarray(wk1_b, f)[sl].reshape(D, 1), (2, 1))),
            "bk2": np.ascontiguousarray(np.tile(np.asarray(wk2_b, f)[sl].reshape(D, 1), (2, 1))),
            "wo1": np.ascontiguousarray(np.asarray(wo1_w, f)[sl, :]).astype(bf),
            "wo2": np.ascontiguousarray(np.asarray(wo2_w, f)[sl, :]).astype(bf),
        })

    res = run_bass_kernel_spmd(nc, in_maps, list(range(NCORES)))
    _cache["last_result"] = res

    out1 = np.zeros((N, C), np.float32)
    out2 = np.zeros((N, C), np.float32)
    for h in range(NCORES):
        rh = res.results[h]
        out1 += (rh["o1pT"] / rh["den1"]).T
        out2 += (rh["o2pT"] / rh["den2"]).T
    # v-bias and output bias constants (v-bias commutes through softmax).
    out1 += np.asarray(wv1_b, f) @ np.asarray(wo1_w, f) + np.asarray(wo1_b, f)
    out2 += np.asarray(wv2_b, f) @ np.asarray(wo2_w, f) + np.asarray(wo2_b, f)
    return out1, out2
